# revision 66
# baseline (speedup 1.0000x reference)
"""Trainium2 Bass kernel for a GPT-style causal attention block.

  y = proj( softmax_causal( (x@Wq)(x@Wk)^T / sqrt(hd) ) @ (x@Wv) )

Shapes (hardcoded): B=2, S=2048, D=1024, H=16 heads, hd=64.

Sharding over 8 NeuronCores: core = (batch b, head-group g), g selects 4
heads. Each core:
  phase 1: QKV projection for its 4 heads (fp32r matmuls).
           q,k produced TRANSPOSED  [head_ch, S]  (contraction-ready),
           v produced natural       [S, head_ch] (+ a ones column), bf16.
           q/k bias adds run on the ACT engine (Identity + per-partition
           bias AP). x arrives pre-transposed in two sequence-half SBUF
           tiles so repeated executions can reload the first half early.
  phase 2: causal attention; BOTH head pairs' jt streams interleaved so
           the per-jt ACT exp latency never stalls the PE, in the
           transposed-score layout [key, query]: the two K=64 score
           matmuls of a pair run CONCURRENTLY in the PE array via
           row-group tile_position (0,0)/(64,0); exp on ACT (scale=1/8
           folded in) writes bf16; causal mask multiplies only the
           diagonal key tiles (bf16 on DVE, 2-4x element rate);
           AV matmul with lhsT=[v|1] bf16 so psum row 64 accumulates
           the softmax denominator; normalization: DVE reciprocal →
           PE-replicate matmul → DVE/ACT bounce → DVE multiply into
           bf16 aT (gpsimd partition_broadcast hangs this runtime
           build, and a pending collective blocks the gpsimd queue).
  phase 3: head/tensor-parallel output projection via ReduceScatter:
           each core computes the PARTIAL projection of its own 256
           channels over the FULL output width, deferred one query
           block so it fills PE bubbles under the next exp stream;
           b_proj/GROUP is folded into the psum->SBUF bounce (DVE,
           bf16); a single ReduceScatter(add) across the 4 cores of
           the batch sums the partials and leaves each core its
           sequence shard of y (collectives cannot write IO tensors,
           so it lands in a DRAM bounce DMA'd to y). One big reduce
           beats chunked overlap on real HW: each collective carries
           ~11us more constant overhead than the cost model's 15us,
           and in repeated execution the tail reduce overlaps the next
           iteration's compute anyway.

Matmuls run in float32r (full PE rate when the moving free dim is
>=256); attention-weight/V/proj matmuls in bf16 (same PE rate). All
host-side sharding/layout prep is data-only so the single SPMD program
is rank-independent.
"""

import numpy as np

B = 2
S = 2048
D = 1024
H = 16
HD = 64
HLOC = 4          # heads per core
NPAIR = 2         # head pairs per core
N_CORES = 8
GROUP = 4         # cores per batch (replica group size)
IB = 512          # query block width (matmul moving dim)
JT = 128          # key tile (psum partition dim)
SCALE = 1.0 / 8.0  # 1/sqrt(hd)
RS_CHUNKS = 1     # ReduceScatter chunks (HW: each collective has ~11us more
                  # constant overhead than the cost model's 15us — one big
                  # reduce beats chunked overlap in steady state)


def _build_bass(s=S, repeat=1, phases="all", rs_chunks=RS_CHUNKS):
    """Build the SPMD Bass program (one NeuronCore's view). `repeat`
    re-runs the whole computation N times inside one NEFF (used to
    measure device time net of dispatch overhead)."""
    import concourse.bacc as bacc
    import concourse.mybir as mybir
    import concourse.tile as tile

    f32 = mybir.dt.float32
    f32r = mybir.dt.float32r
    bf16 = mybir.dt.bfloat16
    Alu = mybir.AluOpType
    Act = mybir.ActivationFunctionType

    n_ib = s // IB           # query blocks
    n_st = s // 128          # 128-row sequence tiles
    n_dt = D // 128          # contraction tiles for D
    n_pt = 256 // 128        # contraction tiles for the local 256 chans

    # Bacc (not plain Bass): its compile() lowers multi-wait sync_infos into
    # event-semaphore nops, which walrus codegen requires.
    nc = bacc.Bacc(num_devices=N_CORES)

    xt = nc.declare_dram_parameter("xt", [D, s], f32r, isOutput=False)
    wqk = nc.declare_dram_parameter("wqk", [D, 512], f32r, isOutput=False)
    wv = nc.declare_dram_parameter("wv", [D, 256], f32r, isOutput=False)
    bqk = nc.declare_dram_parameter("bqk", [128, 4], f32, isOutput=False)
    bv = nc.declare_dram_parameter("bv", [128, 256], f32, isOutput=False)
    # local 256 rows of w_proj, full output width
    wp = nc.declare_dram_parameter("wp", [256, D], bf16, isOutput=False)
    # b_proj / GROUP broadcast over partitions (folded into the psum bounce)
    bp = nc.declare_dram_parameter("bp", [128, D], f32, isOutput=False)
    msk = nc.declare_dram_parameter("msk", [128, 4, IB], bf16, isOutput=False)
    # all-ones constants (f32r memset fails the walrus ISA check, so DMA them)
    one_b = nc.declare_dram_parameter("one_b", [128, 4], bf16, isOutput=False)
    one_r = nc.declare_dram_parameter("one_r", [1, 128], f32r, isOutput=False)
    # selector for the pair-merged reciprocal replicate:
    # sel2[0,0:64]=1, sel2[1,64:128]=1
    sel2 = nc.declare_dram_parameter("sel2", [2, 128], f32r, isOutput=False)
    # bf16 reduce wire: halves collective bytes; host casts y back to f32
    y = nc.declare_dram_parameter("y", [s // GROUP, D], bf16, isOutput=True)

    with tile.TileContext(nc) as tc:
        with (
            tc.tile_pool(name="const", bufs=1) as const,
            tc.tile_pool(name="persist", bufs=1) as persist,
            tc.tile_pool(name="dram", bufs=1, space="DRAM") as dram,
        ):
            bqk_sb = const.tile([128, 4], f32)
            nc.sync.dma_start(out=bqk_sb, in_=bqk[:, :])
            bv_sb = const.tile([128, 256], f32)
            nc.sync.dma_start(out=bv_sb, in_=bv[:, :])
            # msk/wp/bp are not needed until well into the attention phase —
            # their loads are deferred below the first xt chunks
            bp_sb = const.tile([128, D], f32)
            msk_sb = const.tile([128, 4, IB], bf16)
            wp_sb = const.tile([128, n_pt, D], bf16)
            one_r_sb = const.tile([1, 128], f32r)
            nc.sync.dma_start(out=one_r_sb, in_=one_r[:, :])
            sel2_sb = const.tile([2, 128], f32r)
            nc.sync.dma_start(out=sel2_sb, in_=sel2[:, :])
            # dummy exp: pulls the ACT exp table load off the critical path
            warm_sb = const.tile([1, 1], f32)
            nc.scalar.activation(
                out=warm_sb, in_=bqk_sb[0:1, 0:1], func=Act.Exp, scale=0.0
            )

            # persistent intermediates
            for _rep in range(repeat):
             qT_sb = persist.tile([128, NPAIR, s], f32r, name="qT_sb")   # [pair_ch, pair, s]
             kT_sb = persist.tile([128, NPAIR, s], f32r)
             v_sb = persist.tile([128, n_st, HLOC, 65], bf16)  # [:, st, h, 64]=ones
             aT_sb = persist.tile([128, NPAIR, s], bf16)

             rs_in = dram.tile([s, D], bf16, name="rs_in")
             # collectives cannot write IO tensors: reduce into a DRAM
             # bounce, then DMA each chunk into the y output
             rs_out = dram.tile([s // GROUP, D], bf16, name="rs_out")

             for st in range(n_st):
                 nc.sync.dma_start(
                     out=v_sb[:, st, :, 64:65],
                     in_=one_b[:, 0:HLOC].rearrange("p (h o) -> p h o", o=1),
                 )

             # ------- phase 1 + 2: QKV projection interleaved with attention.
             with (
                 tc.tile_pool(name="p1in", bufs=1) as p1in,
             ):
                 # weights first: qkT/v matmuls need ALL of wqk/wv but only
                 # the first sequence-half of xt to get started. wqk split
                 # per c-tile so qkT(0) starts before the rest lands.
                 wqk_sb = p1in.tile([128, n_dt, 512], f32r)
                 for t4 in range(4):
                     nc.sync.dma_start(
                         out=wqk_sb[:, :, t4 * 128 : (t4 + 1) * 128],
                         in_=wqk.rearrange("(t p) c -> p t c", p=128)[
                             :, :, t4 * 128 : (t4 + 1) * 128
                         ],
                     )
                 wv_sb = p1in.tile([128, n_dt, 256], f32r)
                 nc.sync.dma_start(
                     out=wv_sb, in_=wv.rearrange("(t p) c -> p t c", p=128)
                 )
                 # xt in two sequence-half tiles: in repeated execution the
                 # next iteration's first half can reload as soon as its
                 # last reader (early qkT/v blocks) is done, rather than
                 # waiting for the whole tensor's last reader
                 xt_shs = [
                     p1in.tile([128, n_dt, s // 2], f32r, name=f"xt{sh}")
                     for sh in range(2)
                 ]
                 for sh in range(2):
                     for t in range(n_dt):
                         nc.sync.dma_start(
                             out=xt_shs[sh][:, t, :],
                             in_=xt.rearrange("(t p) ss -> p t ss", p=128)[
                                 :, t, sh * s // 2 : (sh + 1) * s // 2
                             ],
                         )
                     if sh == 0 and _rep == 0:
                         # deferred const loads: needed only mid-attention
                         nc.sync.dma_start(out=msk_sb, in_=msk[:, :, :])
                         nc.sync.dma_start(
                             out=wp_sb,
                             in_=wp.rearrange("(t p) c -> p t c", p=128),
                         )
                         nc.sync.dma_start(out=bp_sb, in_=bp[:, :])

                 def xt_seq(dt, lo, width):
                     # [128, width] slice of transposed-x rows dt*128..,
                     # seq cols lo..lo+width (within one sequence half)
                     sh, off = divmod(lo, s // 2)
                     return xt_shs[sh][:, dt, off : off + width]

                 # v natural: lhsT = xT tile [d, s-tile], rhs = Wv [d, 256]
                 def v_for(st_lo, st_hi, pool, eng=None):
                     for st in range(st_lo, st_hi):
                         psv = pool.tile([128, 256], f32, name="psv", tag="pss")
                         for dt in range(n_dt):
                             nc.tensor.matmul(
                                 psv,
                                 lhsT=(xt_seq(dt, st * 128, 128)),
                                 rhs=(wv_sb[:, dt, :]),
                                 start=(dt == 0),
                                 stop=(dt == n_dt - 1),
                             )
                         (eng or nc.vector).tensor_tensor(
                             out=v_sb[:, st, :, 0:64],
                             in0=psv.rearrange("p (h e) -> p h e", h=HLOC),
                             in1=bv_sb.rearrange("p (h e) -> p h e", h=HLOC),
                             op=Alu.add,
                         )

                 def qkT_for(t, sb, pool):
                     # qT/kT: lhsT = W tile [d,c], rhs = xT [d, s-block]
                     # c-tile t: 0,1 = q pair0/1; 2,3 = k pair0/1
                     # bias add on ACT (idle outside the exp stream)
                     ps = pool.tile([128, IB], f32, name="ps", tag="pss")
                     for dt in range(n_dt):
                         nc.tensor.matmul(
                             ps,
                             lhsT=(wqk_sb[:, dt, t * 128 : (t + 1) * 128]),
                             rhs=(xt_seq(dt, sb * IB, IB)),
                             start=(dt == 0),
                             stop=(dt == n_dt - 1),
                         )
                     dst = qT_sb if t < 2 else kT_sb
                     nc.scalar.activation(
                         out=dst[:, t % 2, sb * IB : (sb + 1) * IB],
                         in_=ps,
                         func=Act.Identity,
                         bias=bqk_sb[:, t : t + 1],
                     )

                 if phases == "p1":
                     with tc.tile_pool(
                         name="ps_p1", bufs=2, space="PSUM"
                     ) as ps_p1:
                         v_for(0, n_st, ps_p1)
                         for sb in range(n_ib):
                             for t in range(4):
                                 qkT_for(t, sb, ps_p1)
                     continue
                 # ---- attention: head PAIRS, scores row-group packed ----
                 with (
                     tc.tile_pool(name="ps_s", bufs=2, space="PSUM") as ps_s,
                     tc.tile_pool(name="ps_av", bufs=1, space="PSUM") as ps_av,
                     tc.tile_pool(name="pt", bufs=4) as ptpool,
                     tc.tile_pool(name="small", bufs=4) as small,
                     tc.tile_pool(name="yout", bufs=3) as yout,
                 ):

                  def proj_for(ib, ypool):
                      # partial output projection for seq rows
                      # [ib*IB, (ib+1)*IB): contraction over the local 256
                      # channels (both pairs); b_proj/GROUP folded into the
                      # psum->SBUF bounce (DMA and GPSIMD cannot read PSUM,
                      # ACT is exp-saturated: DVE it is)
                      beng = nc.vector
                      for st in range(4 * ib, 4 * ib + 4):
                          psY = ps_s.tile([128, 2 * IB], f32, name="psY", tag="pss")
                          for half in range(2):
                              hs = slice(half * IB, (half + 1) * IB)
                              for P in range(NPAIR):
                                  nc.tensor.matmul(
                                      psY[:, hs],
                                      lhsT=(aT_sb[:, P, st * 128 : (st + 1) * 128]),
                                      rhs=(wp_sb[:, P, hs]),
                                      start=(P == 0),
                                      stop=(P == NPAIR - 1),
                                  )
                          ysb = ypool.tile([128, D], bf16, name="ysb")
                          beng.tensor_tensor(
                              out=ysb, in0=psY, in1=bp_sb, op=Alu.add
                          )
                          nc.sync.dma_start(
                              out=rs_in[st * 128 : (st + 1) * 128, :], in_=ysb
                          )

                  # uneven chunks: big early chunks hide under the remaining
                  # attention; the last (exposed) chunk is small
                  if rs_chunks == 3:
                      rs_bounds = {1: (0, 2 * IB), 2: (2 * IB, 3 * IB),
                                   3: (3 * IB, 4 * IB)}
                  elif rs_chunks == 2:
                      rs_bounds = {1: (0, 2 * IB), 3: (2 * IB, 4 * IB)}
                  else:
                      rs_bounds = {3: (0, 4 * IB)}

                  def proj_and_rs(ib):
                      proj_for(ib, yout)
                      if ib in rs_bounds:
                          lo, hi = rs_bounds[ib]
                          nc.gpsimd.collective_compute(
                              "ReduceScatter",
                              Alu.add,
                              replica_groups=[[0, 1, 2, 3], [4, 5, 6, 7]],
                              ins=[rs_in[lo:hi, :]],
                              outs=[rs_out[lo // GROUP : hi // GROUP, :]],
                          )
                          nc.sync.dma_start(
                              out=y[lo // GROUP : hi // GROUP, :],
                              in_=rs_out[lo // GROUP : hi // GROUP, :],
                          )

                  # q/k for BOTH pairs upfront (sb-major so early blocks
                  # only need the first xt sequence-half)
                  for sb in range(n_ib):
                      for t in range(4):
                          qkT_for(t, sb, ps_s)

                  # Both pairs' jt streams INTERLEAVED: while one pair's exp
                  # runs on ACT, the PE works the other pair's scores/AV —
                  # the per-jt exp latency never stalls the PE. v tiles load
                  # once per ib and feed both pairs.
                  pending = None   # proj deferred one ib into the next stream
                  for ib in range(n_ib):
                     v_for(4 * ib, 4 * ib + 4, ps_s)
                     njt = 4 * (ib + 1)  # key tiles needed (j <= i)
                     avs = {
                         (pair, hh): ps_av.tile(
                             [65, IB], f32, name=f"av{pair}{hh}",
                             tag=f"av{pair}{hh}",
                         )
                         for pair in range(NPAIR)
                         for hh in range(2)
                     }
                     # diagonal key tiles first: their mask multiply then
                     # overlaps the long non-diagonal score/AV stream
                     jt_order = list(range(4 * ib, njt)) + list(range(4 * ib))
                     for jseq, jt in enumerate(jt_order):
                         for pair in range(NPAIR):
                             pss = ps_s.tile([128, 2 * IB], f32, name="pss")
                             for hh in range(2):
                                 off = hh * 64
                                 nc.tensor.matmul(
                                     pss[:, hh * IB : (hh + 1) * IB],
                                     lhsT=(kT_sb[
                                             off : off + 64,
                                             pair,
                                             jt * 128 : (jt + 1) * 128,
                                         ]
                                     ),
                                     rhs=(qT_sb[
                                             off : off + 64,
                                             pair,
                                             ib * IB : (ib + 1) * IB,
                                         ]
                                     ),
                                     start=True,
                                     stop=True,
                                     tile_position=(off, 0),
                                 )
                             pt = ptpool.tile([128, 2 * IB], bf16, name="pt")
                             nc.scalar.activation(
                                 out=pt, in_=pss, func=Act.Exp, scale=SCALE
                             )
                             k = jt - 4 * ib
                             for hh in range(2):
                                 if k >= 0:  # diagonal tile: causal mask
                                     # DVE only: Pool would queue these
                                     # behind the previous rep's collective
                                     nc.vector.tensor_tensor(
                                         out=pt[:, hh * IB : (hh + 1) * IB],
                                         in0=pt[:, hh * IB : (hh + 1) * IB],
                                         in1=msk_sb[:, k, :],
                                         op=Alu.mult,
                                     )
                                 nc.tensor.matmul(
                                     avs[pair, hh],
                                     lhsT=(v_sb[:, jt, pair * 2 + hh, :]),
                                     rhs=(pt[:, hh * IB : (hh + 1) * IB]),
                                     start=(jseq == 0),
                                     stop=(jseq == njt - 1),
                                 )
                     if phases == "all" and pending is not None:
                         # previous ib's partial projection + reduce, emitted
                         # after this ib's whole jt stream: maximum slack for
                         # the previous normalize chain to drain on DVE
                         proj_and_rs(pending)
                         pending = None
                     # normalize per head: aT = av[0:64] * (1 / av[64]).
                     # Both heads of a pair share ONE replicate matmul
                     # (sel2 maps rec row hh to partitions hh*64..) and one
                     # psum->SBUF bounce (gpsimd partition_broadcast hangs
                     # this runtime, so replicate via PE).
                     # (matmul psum output must start at partition 0 — the
                     # pair-merged replicate into one [128,·] psum fails the
                     # walrus ISA check, so replicate per head)
                     for pair in range(NPAIR):
                         for hh in range(2):
                             av = avs[pair, hh]
                             off = hh * 64
                             rec_sb = small.tile([1, IB], f32r, name="rec_sb")
                             with nc.allow_low_precision(
                                 reason="softmax denom reciprocal, f32r"
                             ):
                                 nc.vector.reciprocal(rec_sb, av[64:65, :])
                             rec_ps = ps_s.tile(
                                 [64, IB], f32, name="rec_ps", tag="pss"
                             )
                             nc.tensor.matmul(
                                 rec_ps,
                                 lhsT=(one_r_sb[0:1, 0:64]),
                                 rhs=(rec_sb[:, :]),
                                 start=True,
                                 stop=True,
                             )
                             rec_rep = small.tile(
                                 [64, IB], f32, name="rec_rep"
                             )
                             if pair == 0:
                                 nc.vector.tensor_copy(
                                     out=rec_rep, in_=rec_ps
                                 )
                             else:
                                 # ACT has slack at ib boundaries
                                 nc.scalar.activation(
                                     out=rec_rep, in_=rec_ps, func=Act.Copy
                                 )
                             nc.vector.tensor_tensor(
                                 out=aT_sb[
                                     off : off + 64, pair, ib * IB : (ib + 1) * IB
                                 ],
                                 in0=av[0:64, :],
                                 in1=rec_rep,
                                 op=Alu.mult,
                             )
                     pending = ib
                  if phases == "all" and pending is not None:
                      proj_and_rs(pending)

    nc.compile()
    return nc


def _shard_inputs(x, w_attn, b_attn, w_proj, b_proj, s=S):
    """Host-side sharding: build the per-core input maps."""
    import ml_dtypes
    x = np.asarray(x, dtype=np.float32)
    w_attn = np.asarray(w_attn, dtype=np.float32)
    b_attn = np.asarray(b_attn, dtype=np.float32)
    w_proj = np.asarray(w_proj, dtype=np.float32)
    b_proj = np.asarray(b_proj, dtype=np.float32)

    # causal mask tiles: msk[j, k, i] = 1.0 if i >= j + 128*k
    jj = np.arange(128)[:, None, None]
    kk = np.arange(4)[None, :, None]
    ii = np.arange(IB)[None, None, :]
    msk = (ii >= jj + 128 * kk).astype(ml_dtypes.bfloat16)

    in_maps = []
    for core in range(N_CORES):
        b, g = divmod(core, GROUP)
        hs = list(range(g * HLOC, (g + 1) * HLOC))
        xt = np.ascontiguousarray(x[b].T)
        qcols = np.concatenate(
            [w_attn[:, h * HD : (h + 1) * HD] for h in hs], axis=1
        )
        kcols = np.concatenate(
            [w_attn[:, D + h * HD : D + (h + 1) * HD] for h in hs], axis=1
        )
        vcols = np.concatenate(
            [w_attn[:, 2 * D + h * HD : 2 * D + (h + 1) * HD] for h in hs], axis=1
        )
        wqk = np.ascontiguousarray(np.concatenate([qcols, kcols], axis=1))
        wvv = np.ascontiguousarray(vcols)
        bq = np.concatenate([b_attn[h * HD : (h + 1) * HD] for h in hs])
        bk = np.concatenate([b_attn[D + h * HD : D + (h + 1) * HD] for h in hs])
        bvv = np.concatenate(
            [b_attn[2 * D + h * HD : 2 * D + (h + 1) * HD] for h in hs]
        )
        bqk = np.concatenate([bq, bk]).reshape(4, 128).T.copy()  # [128, 4]
        bv = np.broadcast_to(bvv, (128, 256)).copy()
        wpc = np.ascontiguousarray(
            w_proj[g * 256 : (g + 1) * 256, :]
        ).astype(ml_dtypes.bfloat16)
        bpc = np.broadcast_to(b_proj / GROUP, (128, D)).astype(np.float32).copy()
        in_maps.append(
            dict(
                xt=xt, wqk=wqk, wv=wvv, bqk=bqk, bv=bv, wp=wpc, bp=bpc,
                msk=msk,
                one_b=np.ones((128, 4), ml_dtypes.bfloat16),
                one_r=np.ones((1, 128), np.float32),
                sel2=np.repeat(np.eye(2, dtype=np.float32), 64, axis=1),
            )
        )
    return in_maps


_BOUNDS_BY_CHUNKS = {
    3: [(0, 2 * IB), (2 * IB, 3 * IB), (3 * IB, 4 * IB)],
    2: [(0, 2 * IB), (2 * IB, 4 * IB)],
    1: [(0, 4 * IB)],
}
RS_BOUNDS = _BOUNDS_BY_CHUNKS[RS_CHUNKS]


def _unshard(results):
    y = np.empty((B, S, D), np.float32)
    for core in range(N_CORES):
        b, g = divmod(core, GROUP)
        res = results[core]["y"]
        for lo, hi in RS_BOUNDS:
            n = (hi - lo) // GROUP
            y[b, lo + g * n : lo + (g + 1) * n, :] = res[
                lo // GROUP : hi // GROUP
            ]
    return y


_NC_CACHE = {}


def kernel(x, w_attn, b_attn, w_proj, b_proj):
    from concourse.bass_utils import run_bass_kernel_spmd

    if S not in _NC_CACHE:
        _NC_CACHE[S] = _build_bass(S)
    nc = _NC_CACHE[S]
    in_maps = _shard_inputs(x, w_attn, b_attn, w_proj, b_proj)
    res = run_bass_kernel_spmd(nc, in_maps, list(range(N_CORES)))
    return _unshard(res.results)


# revision 68
# speedup vs baseline: 1.0644x; 1.0644x over previous
"""Trainium2 Bass kernel for a GPT-style causal attention block.

  y = proj( softmax_causal( (x@Wq)(x@Wk)^T / sqrt(hd) ) @ (x@Wv) )

Shapes (hardcoded): B=2, S=2048, D=1024, H=16 heads, hd=64.

Sharding over 8 NeuronCores: core = (batch b, head-group g), g selects 4
heads. Each core:
  phase 1: QKV projection for its 4 heads (fp32r matmuls).
           q,k produced TRANSPOSED  [head_ch, S]  (contraction-ready),
           v produced natural       [S, head_ch] (+ a ones column), bf16.
           q/k bias adds run on the ACT engine (Identity + per-partition
           bias AP). x arrives pre-transposed in two sequence-half SBUF
           tiles so repeated executions can reload the first half early.
  phase 2: causal attention; BOTH head pairs' jt streams interleaved so
           the per-jt ACT exp latency never stalls the PE, in the
           transposed-score layout [key, query]: the two K=64 score
           matmuls of a pair run CONCURRENTLY in the PE array via
           row-group tile_position (0,0)/(64,0); exp on ACT (scale=1/8
           folded in) writes bf16; causal mask multiplies only the
           diagonal key tiles (bf16 on DVE, 2-4x element rate);
           AV matmul with lhsT=[v|1] bf16 so psum row 64 accumulates
           the softmax denominator; normalization: DVE reciprocal →
           PE-replicate matmul → DVE/ACT bounce → DVE multiply into
           bf16 aT (gpsimd partition_broadcast hangs this runtime
           build, and a pending collective blocks the gpsimd queue).
  phase 3: head/tensor-parallel output projection via ReduceScatter:
           each core computes the PARTIAL projection of its own 256
           channels over the FULL output width, deferred one query
           block so it fills PE bubbles under the next exp stream;
           b_proj/GROUP is folded into the psum->SBUF bounce (DVE,
           bf16); a single ReduceScatter(add) across the 4 cores of
           the batch sums the partials and leaves each core its
           sequence shard of y (collectives cannot write IO tensors,
           so it lands in a DRAM bounce DMA'd to y). One big reduce
           beats chunked overlap on real HW: each collective carries
           ~11us more constant overhead than the cost model's 15us,
           and in repeated execution the tail reduce overlaps the next
           iteration's compute anyway.

Matmuls run in float32r (full PE rate when the moving free dim is
>=256); attention-weight/V/proj matmuls in bf16 (same PE rate). All
host-side sharding/layout prep is data-only so the single SPMD program
is rank-independent.
"""

import numpy as np

B = 2
S = 2048
D = 1024
H = 16
HD = 64
HLOC = 4          # heads per core
NPAIR = 2         # head pairs per core
N_CORES = 8
GROUP = 4         # cores per batch (replica group size)
IB = 512          # query block width (matmul moving dim)
JT = 128          # key tile (psum partition dim)
SCALE = 1.0 / 8.0  # 1/sqrt(hd)
RS_CHUNKS = 1     # ReduceScatter chunks (HW: each collective has ~11us more
                  # constant overhead than the cost model's 15us — one big
                  # reduce beats chunked overlap in steady state)


def _build_bass(s=S, repeat=1, phases="all", rs_chunks=RS_CHUNKS):
    """Build the SPMD Bass program (one NeuronCore's view). `repeat`
    re-runs the whole computation N times inside one NEFF (used to
    measure device time net of dispatch overhead)."""
    import concourse.bacc as bacc
    import concourse.mybir as mybir
    import concourse.tile as tile

    f32 = mybir.dt.float32
    f32r = mybir.dt.float32r
    bf16 = mybir.dt.bfloat16
    Alu = mybir.AluOpType
    Act = mybir.ActivationFunctionType

    n_ib = s // IB           # query blocks
    n_st = s // 128          # 128-row sequence tiles
    n_dt = D // 128          # contraction tiles for D
    n_pt = 256 // 128        # contraction tiles for the local 256 chans

    # Bacc (not plain Bass): its compile() lowers multi-wait sync_infos into
    # event-semaphore nops, which walrus codegen requires.
    nc = bacc.Bacc(num_devices=N_CORES)

    xt = nc.declare_dram_parameter("xt", [D, s], f32r, isOutput=False)
    wqk = nc.declare_dram_parameter("wqk", [D, 512], f32r, isOutput=False)
    wv = nc.declare_dram_parameter("wv", [D, 256], f32r, isOutput=False)
    bqk = nc.declare_dram_parameter("bqk", [128, 4], f32, isOutput=False)
    bv = nc.declare_dram_parameter("bv", [128, 256], f32, isOutput=False)
    # local 256 rows of w_proj, full output width
    wp = nc.declare_dram_parameter("wp", [256, D], bf16, isOutput=False)
    # b_proj / GROUP broadcast over partitions (folded into the psum bounce)
    bp = nc.declare_dram_parameter("bp", [128, D], f32, isOutput=False)
    msk = nc.declare_dram_parameter("msk", [128, 4, IB], bf16, isOutput=False)
    # all-ones constants (f32r memset fails the walrus ISA check, so DMA them)
    one_b = nc.declare_dram_parameter("one_b", [128, 4], bf16, isOutput=False)
    one_r = nc.declare_dram_parameter("one_r", [1, 128], f32r, isOutput=False)
    # selector for the pair-merged reciprocal replicate:
    # sel2[0,0:64]=1, sel2[1,64:128]=1
    sel2 = nc.declare_dram_parameter("sel2", [2, 128], f32r, isOutput=False)
    # bf16 reduce wire: halves collective bytes; host casts y back to f32
    y = nc.declare_dram_parameter("y", [s // GROUP, D], bf16, isOutput=True)

    with tile.TileContext(nc) as tc:
        with (
            tc.tile_pool(name="const", bufs=1) as const,
            tc.tile_pool(name="persist", bufs=1) as persist,
            tc.tile_pool(name="dram", bufs=1, space="DRAM") as dram,
        ):
            bqk_sb = const.tile([128, 4], f32)
            nc.sync.dma_start(out=bqk_sb, in_=bqk[:, :])
            bv_sb = const.tile([128, 256], f32)
            nc.sync.dma_start(out=bv_sb, in_=bv[:, :])
            # msk/wp/bp are not needed until well into the attention phase —
            # their loads are deferred below the first xt chunks
            bp_sb = const.tile([128, D], f32)
            msk_sb = const.tile([128, 4, IB], bf16)
            wp_sb = const.tile([128, n_pt, D], bf16)
            one_r_sb = const.tile([1, 128], f32r)
            nc.sync.dma_start(out=one_r_sb, in_=one_r[:, :])
            sel2_sb = const.tile([2, 128], f32r)
            nc.sync.dma_start(out=sel2_sb, in_=sel2[:, :])
            # dummy exp: pulls the ACT exp table load off the critical path
            warm_sb = const.tile([1, 1], f32)
            nc.scalar.activation(
                out=warm_sb, in_=bqk_sb[0:1, 0:1], func=Act.Exp, scale=0.0
            )

            # persistent intermediates
            for _rep in range(repeat):
             qT_sb = persist.tile([128, NPAIR, s], f32r, name="qT_sb")   # [pair_ch, pair, s]
             kT_sb = persist.tile([128, NPAIR, s], f32r)
             v_sb = persist.tile([128, n_st, HLOC, 65], bf16)  # [:, st, h, 64]=ones
             aT_sb = persist.tile([128, NPAIR, s], bf16)

             rs_in = dram.tile([s, D], bf16, name="rs_in")
             # collectives cannot write IO tensors: reduce into a DRAM
             # bounce, then DMA each chunk into the y output
             rs_out = dram.tile([s // GROUP, D], bf16, name="rs_out")

             for st in range(n_st):
                 nc.sync.dma_start(
                     out=v_sb[:, st, :, 64:65],
                     in_=one_b[:, 0:HLOC].rearrange("p (h o) -> p h o", o=1),
                 )

             # ------- phase 1 + 2: QKV projection interleaved with attention.
             with (
                 tc.tile_pool(name="p1in", bufs=1) as p1in,
             ):
                 # weights first: qkT/v matmuls need ALL of wqk/wv but only
                 # the first sequence-half of xt to get started. wqk split
                 # per c-tile so qkT(0) starts before the rest lands.
                 wqk_sb = p1in.tile([128, n_dt, 512], f32r)
                 for t4 in range(4):
                     nc.sync.dma_start(
                         out=wqk_sb[:, :, t4 * 128 : (t4 + 1) * 128],
                         in_=wqk.rearrange("(t p) c -> p t c", p=128)[
                             :, :, t4 * 128 : (t4 + 1) * 128
                         ],
                     )
                 wv_sb = p1in.tile([128, n_dt, 256], f32r)
                 for t4 in range(2):
                     nc.sync.dma_start(
                         out=wv_sb[:, :, t4 * 128 : (t4 + 1) * 128],
                         in_=wv.rearrange("(t p) c -> p t c", p=128)[
                             :, :, t4 * 128 : (t4 + 1) * 128
                         ],
                     )
                 # xt in two sequence-half tiles: in repeated execution the
                 # next iteration's first half can reload as soon as its
                 # last reader (early qkT/v blocks) is done, rather than
                 # waiting for the whole tensor's last reader
                 xt_shs = [
                     p1in.tile([128, n_dt, s // 2], f32r, name=f"xt{sh}")
                     for sh in range(2)
                 ]
                 for sh in range(2):
                     for t in range(n_dt):
                         for q in range(2):
                             # half-chunks spread across more DMA rings
                             # (real HW has 16; the model's 8 see no change)
                             qs = slice(q * s // 4, (q + 1) * s // 4)
                             nc.sync.dma_start(
                                 out=xt_shs[sh][:, t, qs],
                                 in_=xt.rearrange("(t p) ss -> p t ss", p=128)[
                                     :, t, sh * s // 2 + q * s // 4 :
                                     sh * s // 2 + (q + 1) * s // 4
                                 ],
                             )
                     if sh == 0 and _rep == 0:
                         # deferred const loads: needed only mid-attention
                         nc.sync.dma_start(out=msk_sb, in_=msk[:, :, :])
                         nc.sync.dma_start(
                             out=wp_sb,
                             in_=wp.rearrange("(t p) c -> p t c", p=128),
                         )
                         nc.sync.dma_start(out=bp_sb, in_=bp[:, :])

                 def xt_seq(dt, lo, width):
                     # [128, width] slice of transposed-x rows dt*128..,
                     # seq cols lo..lo+width (within one sequence half)
                     sh, off = divmod(lo, s // 2)
                     return xt_shs[sh][:, dt, off : off + width]

                 # v natural: lhsT = xT tile [d, s-tile], rhs = Wv [d, 256]
                 def v_for(st_lo, st_hi, pool, eng=None):
                     for st in range(st_lo, st_hi):
                         psv = pool.tile([128, 256], f32, name="psv", tag="pss")
                         for dt in range(n_dt):
                             nc.tensor.matmul(
                                 psv,
                                 lhsT=(xt_seq(dt, st * 128, 128)),
                                 rhs=(wv_sb[:, dt, :]),
                                 start=(dt == 0),
                                 stop=(dt == n_dt - 1),
                             )
                         (eng or nc.vector).tensor_tensor(
                             out=v_sb[:, st, :, 0:64],
                             in0=psv.rearrange("p (h e) -> p h e", h=HLOC),
                             in1=bv_sb.rearrange("p (h e) -> p h e", h=HLOC),
                             op=Alu.add,
                         )

                 def qkT_for(t, sb, pool):
                     # qT/kT: lhsT = W tile [d,c], rhs = xT [d, s-block]
                     # c-tile t: 0,1 = q pair0/1; 2,3 = k pair0/1
                     # bias add on ACT (idle outside the exp stream)
                     ps = pool.tile([128, IB], f32, name="ps", tag="pss")
                     for dt in range(n_dt):
                         nc.tensor.matmul(
                             ps,
                             lhsT=(wqk_sb[:, dt, t * 128 : (t + 1) * 128]),
                             rhs=(xt_seq(dt, sb * IB, IB)),
                             start=(dt == 0),
                             stop=(dt == n_dt - 1),
                         )
                     dst = qT_sb if t < 2 else kT_sb
                     nc.scalar.activation(
                         out=dst[:, t % 2, sb * IB : (sb + 1) * IB],
                         in_=ps,
                         func=Act.Identity,
                         bias=bqk_sb[:, t : t + 1],
                     )

                 if phases == "p1":
                     with tc.tile_pool(
                         name="ps_p1", bufs=2, space="PSUM"
                     ) as ps_p1:
                         v_for(0, n_st, ps_p1)
                         for sb in range(n_ib):
                             for t in range(4):
                                 qkT_for(t, sb, ps_p1)
                     continue
                 # ---- attention: head PAIRS, scores row-group packed ----
                 with (
                     tc.tile_pool(name="ps_s", bufs=2, space="PSUM") as ps_s,
                     tc.tile_pool(name="ps_av", bufs=1, space="PSUM") as ps_av,
                     tc.tile_pool(name="pt", bufs=4) as ptpool,
                     tc.tile_pool(name="small", bufs=4) as small,
                     tc.tile_pool(name="yout", bufs=3) as yout,
                 ):

                  def proj_for(ib, ypool):
                      # partial output projection for seq rows
                      # [ib*IB, (ib+1)*IB): contraction over the local 256
                      # channels (both pairs); b_proj/GROUP folded into the
                      # psum->SBUF bounce (DMA and GPSIMD cannot read PSUM,
                      # ACT is exp-saturated: DVE it is)
                      beng = nc.vector
                      for st in range(4 * ib, 4 * ib + 4):
                          psY = ps_s.tile([128, 2 * IB], f32, name="psY", tag="pss")
                          for half in range(2):
                              hs = slice(half * IB, (half + 1) * IB)
                              for P in range(NPAIR):
                                  nc.tensor.matmul(
                                      psY[:, hs],
                                      lhsT=(aT_sb[:, P, st * 128 : (st + 1) * 128]),
                                      rhs=(wp_sb[:, P, hs]),
                                      start=(P == 0),
                                      stop=(P == NPAIR - 1),
                                  )
                          ysb = ypool.tile([128, D], bf16, name="ysb")
                          beng.tensor_tensor(
                              out=ysb, in0=psY, in1=bp_sb, op=Alu.add
                          )
                          nc.sync.dma_start(
                              out=rs_in[st * 128 : (st + 1) * 128, :], in_=ysb
                          )

                  # uneven chunks: big early chunks hide under the remaining
                  # attention; the last (exposed) chunk is small
                  if rs_chunks == 3:
                      rs_bounds = {1: (0, 2 * IB), 2: (2 * IB, 3 * IB),
                                   3: (3 * IB, 4 * IB)}
                  elif rs_chunks == 2:
                      rs_bounds = {1: (0, 2 * IB), 3: (2 * IB, 4 * IB)}
                  else:
                      rs_bounds = {3: (0, 4 * IB)}

                  def proj_and_rs(ib):
                      proj_for(ib, yout)
                      if ib in rs_bounds:
                          lo, hi = rs_bounds[ib]
                          nc.gpsimd.collective_compute(
                              "ReduceScatter",
                              Alu.add,
                              replica_groups=[[0, 1, 2, 3], [4, 5, 6, 7]],
                              ins=[rs_in[lo:hi, :]],
                              outs=[rs_out[lo // GROUP : hi // GROUP, :]],
                          )
                          nc.sync.dma_start(
                              out=y[lo // GROUP : hi // GROUP, :],
                              in_=rs_out[lo // GROUP : hi // GROUP, :],
                          )

                  # q/k for BOTH pairs upfront (sb-major so early blocks
                  # only need the first xt sequence-half)
                  for sb in range(n_ib):
                      for t in range(4):
                          qkT_for(t, sb, ps_s)

                  # Both pairs' jt streams INTERLEAVED: while one pair's exp
                  # runs on ACT, the PE works the other pair's scores/AV —
                  # the per-jt exp latency never stalls the PE. v tiles load
                  # once per ib and feed both pairs.
                  pending = None   # proj deferred one ib into the next stream
                  for ib in range(n_ib):
                     v_for(4 * ib, 4 * ib + 4, ps_s)
                     njt = 4 * (ib + 1)  # key tiles needed (j <= i)
                     avs = {
                         (pair, hh): ps_av.tile(
                             [65, IB], f32, name=f"av{pair}{hh}",
                             tag=f"av{pair}{hh}",
                         )
                         for pair in range(NPAIR)
                         for hh in range(2)
                     }
                     # diagonal key tiles first: their mask multiply then
                     # overlaps the long non-diagonal score/AV stream
                     jt_order = list(range(4 * ib, njt)) + list(range(4 * ib))
                     for jseq, jt in enumerate(jt_order):
                         for pair in range(NPAIR):
                             pss = ps_s.tile([128, 2 * IB], f32, name="pss")
                             for hh in range(2):
                                 off = hh * 64
                                 nc.tensor.matmul(
                                     pss[:, hh * IB : (hh + 1) * IB],
                                     lhsT=(kT_sb[
                                             off : off + 64,
                                             pair,
                                             jt * 128 : (jt + 1) * 128,
                                         ]
                                     ),
                                     rhs=(qT_sb[
                                             off : off + 64,
                                             pair,
                                             ib * IB : (ib + 1) * IB,
                                         ]
                                     ),
                                     start=True,
                                     stop=True,
                                     tile_position=(off, 0),
                                 )
                             pt = ptpool.tile([128, 2 * IB], bf16, name="pt")
                             nc.scalar.activation(
                                 out=pt, in_=pss, func=Act.Exp, scale=SCALE
                             )
                             k = jt - 4 * ib
                             for hh in range(2):
                                 if k >= 0:  # diagonal tile: causal mask
                                     # DVE only: Pool would queue these
                                     # behind the previous rep's collective
                                     nc.vector.tensor_tensor(
                                         out=pt[:, hh * IB : (hh + 1) * IB],
                                         in0=pt[:, hh * IB : (hh + 1) * IB],
                                         in1=msk_sb[:, k, :],
                                         op=Alu.mult,
                                     )
                                 nc.tensor.matmul(
                                     avs[pair, hh],
                                     lhsT=(v_sb[:, jt, pair * 2 + hh, :]),
                                     rhs=(pt[:, hh * IB : (hh + 1) * IB]),
                                     start=(jseq == 0),
                                     stop=(jseq == njt - 1),
                                 )
                     if phases == "all" and pending is not None:
                         # previous ib's partial projection + reduce, emitted
                         # after this ib's whole jt stream: maximum slack for
                         # the previous normalize chain to drain on DVE
                         proj_and_rs(pending)
                         pending = None
                     # normalize per head: aT = av[0:64] * (1 / av[64]).
                     # Both heads of a pair share ONE replicate matmul
                     # (sel2 maps rec row hh to partitions hh*64..) and one
                     # psum->SBUF bounce (gpsimd partition_broadcast hangs
                     # this runtime, so replicate via PE).
                     # (matmul psum output must start at partition 0 — the
                     # pair-merged replicate into one [128,·] psum fails the
                     # walrus ISA check, so replicate per head)
                     for pair in range(NPAIR):
                         for hh in range(2):
                             av = avs[pair, hh]
                             off = hh * 64
                             rec_sb = small.tile([1, IB], f32r, name="rec_sb")
                             with nc.allow_low_precision(
                                 reason="softmax denom reciprocal, f32r"
                             ):
                                 nc.vector.reciprocal(rec_sb, av[64:65, :])
                             rec_ps = ps_s.tile(
                                 [64, IB], f32, name="rec_ps", tag="pss"
                             )
                             nc.tensor.matmul(
                                 rec_ps,
                                 lhsT=(one_r_sb[0:1, 0:64]),
                                 rhs=(rec_sb[:, :]),
                                 start=True,
                                 stop=True,
                             )
                             rec_rep = small.tile(
                                 [64, IB], f32, name="rec_rep"
                             )
                             if pair == 0:
                                 nc.vector.tensor_copy(
                                     out=rec_rep, in_=rec_ps
                                 )
                             else:
                                 # ACT has slack at ib boundaries
                                 nc.scalar.activation(
                                     out=rec_rep, in_=rec_ps, func=Act.Copy
                                 )
                             nc.vector.tensor_tensor(
                                 out=aT_sb[
                                     off : off + 64, pair, ib * IB : (ib + 1) * IB
                                 ],
                                 in0=av[0:64, :],
                                 in1=rec_rep,
                                 op=Alu.mult,
                             )
                     pending = ib
                  if phases == "all" and pending is not None:
                      proj_and_rs(pending)

    nc.compile()
    return nc


def _shard_inputs(x, w_attn, b_attn, w_proj, b_proj, s=S):
    """Host-side sharding: build the per-core input maps."""
    import ml_dtypes
    x = np.asarray(x, dtype=np.float32)
    w_attn = np.asarray(w_attn, dtype=np.float32)
    b_attn = np.asarray(b_attn, dtype=np.float32)
    w_proj = np.asarray(w_proj, dtype=np.float32)
    b_proj = np.asarray(b_proj, dtype=np.float32)

    # causal mask tiles: msk[j, k, i] = 1.0 if i >= j + 128*k
    jj = np.arange(128)[:, None, None]
    kk = np.arange(4)[None, :, None]
    ii = np.arange(IB)[None, None, :]
    msk = (ii >= jj + 128 * kk).astype(ml_dtypes.bfloat16)

    in_maps = []
    for core in range(N_CORES):
        b, g = divmod(core, GROUP)
        hs = list(range(g * HLOC, (g + 1) * HLOC))
        xt = np.ascontiguousarray(x[b].T)
        qcols = np.concatenate(
            [w_attn[:, h * HD : (h + 1) * HD] for h in hs], axis=1
        )
        kcols = np.concatenate(
            [w_attn[:, D + h * HD : D + (h + 1) * HD] for h in hs], axis=1
        )
        vcols = np.concatenate(
            [w_attn[:, 2 * D + h * HD : 2 * D + (h + 1) * HD] for h in hs], axis=1
        )
        wqk = np.ascontiguousarray(np.concatenate([qcols, kcols], axis=1))
        wvv = np.ascontiguousarray(vcols)
        bq = np.concatenate([b_attn[h * HD : (h + 1) * HD] for h in hs])
        bk = np.concatenate([b_attn[D + h * HD : D + (h + 1) * HD] for h in hs])
        bvv = np.concatenate(
            [b_attn[2 * D + h * HD : 2 * D + (h + 1) * HD] for h in hs]
        )
        bqk = np.concatenate([bq, bk]).reshape(4, 128).T.copy()  # [128, 4]
        bv = np.broadcast_to(bvv, (128, 256)).copy()
        wpc = np.ascontiguousarray(
            w_proj[g * 256 : (g + 1) * 256, :]
        ).astype(ml_dtypes.bfloat16)
        bpc = np.broadcast_to(b_proj / GROUP, (128, D)).astype(np.float32).copy()
        in_maps.append(
            dict(
                xt=xt, wqk=wqk, wv=wvv, bqk=bqk, bv=bv, wp=wpc, bp=bpc,
                msk=msk,
                one_b=np.ones((128, 4), ml_dtypes.bfloat16),
                one_r=np.ones((1, 128), np.float32),
                sel2=np.repeat(np.eye(2, dtype=np.float32), 64, axis=1),
            )
        )
    return in_maps


_BOUNDS_BY_CHUNKS = {
    3: [(0, 2 * IB), (2 * IB, 3 * IB), (3 * IB, 4 * IB)],
    2: [(0, 2 * IB), (2 * IB, 4 * IB)],
    1: [(0, 4 * IB)],
}
RS_BOUNDS = _BOUNDS_BY_CHUNKS[RS_CHUNKS]


def _unshard(results):
    y = np.empty((B, S, D), np.float32)
    for core in range(N_CORES):
        b, g = divmod(core, GROUP)
        res = results[core]["y"]
        for lo, hi in RS_BOUNDS:
            n = (hi - lo) // GROUP
            y[b, lo + g * n : lo + (g + 1) * n, :] = res[
                lo // GROUP : hi // GROUP
            ]
    return y


_NC_CACHE = {}


def kernel(x, w_attn, b_attn, w_proj, b_proj):
    from concourse.bass_utils import run_bass_kernel_spmd

    if S not in _NC_CACHE:
        _NC_CACHE[S] = _build_bass(S)
    nc = _NC_CACHE[S]
    in_maps = _shard_inputs(x, w_attn, b_attn, w_proj, b_proj)
    res = run_bass_kernel_spmd(nc, in_maps, list(range(N_CORES)))
    return _unshard(res.results)


# revision 71
# speedup vs baseline: 1.0720x; 1.0072x over previous
"""Trainium2 Bass kernel for a GPT-style causal attention block.

  y = proj( softmax_causal( (x@Wq)(x@Wk)^T / sqrt(hd) ) @ (x@Wv) )

Shapes (hardcoded): B=2, S=2048, D=1024, H=16 heads, hd=64.

Sharding over 8 NeuronCores: core = (batch b, head-group g), g selects 4
heads. Each core:
  phase 1: QKV projection for its 4 heads (fp32r matmuls).
           q,k produced TRANSPOSED  [head_ch, S]  (contraction-ready),
           v produced natural       [S, head_ch] (+ a ones column), bf16.
           q/k bias adds run on the ACT engine (Identity + per-partition
           bias AP). x arrives pre-transposed in two sequence-half SBUF
           tiles so repeated executions can reload the first half early.
  phase 2: causal attention; BOTH head pairs' jt streams interleaved so
           the per-jt ACT exp latency never stalls the PE, in the
           transposed-score layout [key, query]: the two K=64 score
           matmuls of a pair run CONCURRENTLY in the PE array via
           row-group tile_position (0,0)/(64,0); exp on ACT (scale=1/8
           folded in) writes bf16; causal mask multiplies only the
           diagonal key tiles (bf16 on DVE, 2-4x element rate);
           AV matmul with lhsT=[v|1] bf16 so psum row 64 accumulates
           the softmax denominator; normalization: DVE reciprocal →
           PE-replicate matmul → DVE/ACT bounce → DVE multiply into
           bf16 aT (gpsimd partition_broadcast hangs this runtime
           build, and a pending collective blocks the gpsimd queue).
  phase 3: head/tensor-parallel output projection via ReduceScatter:
           each core computes the PARTIAL projection of its own 256
           channels over the FULL output width, deferred one query
           block so it fills PE bubbles under the next exp stream;
           b_proj/GROUP is folded into the psum->SBUF bounce (DVE,
           bf16); a single ReduceScatter(add) across the 4 cores of
           the batch sums the partials and leaves each core its
           sequence shard of y (collectives cannot write IO tensors,
           so it lands in a DRAM bounce DMA'd to y). One big reduce
           beats chunked overlap on real HW: each collective carries
           ~11us more constant overhead than the cost model's 15us,
           and in repeated execution the tail reduce overlaps the next
           iteration's compute anyway.

Matmuls run in float32r (full PE rate when the moving free dim is
>=256); attention-weight/V/proj matmuls in bf16 (same PE rate). All
host-side sharding/layout prep is data-only so the single SPMD program
is rank-independent.
"""

import numpy as np

B = 2
S = 2048
D = 1024
H = 16
HD = 64
HLOC = 4          # heads per core
NPAIR = 2         # head pairs per core
N_CORES = 8
GROUP = 4         # cores per batch (replica group size)
IB = 512          # query block width (matmul moving dim)
JT = 128          # key tile (psum partition dim)
SCALE = 1.0 / 8.0  # 1/sqrt(hd)
RS_CHUNKS = 1     # ReduceScatter chunks (HW: each collective has ~11us more
                  # constant overhead than the cost model's 15us — one big
                  # reduce beats chunked overlap in steady state)


def _build_bass(s=S, repeat=1, phases="all", rs_chunks=RS_CHUNKS):
    """Build the SPMD Bass program (one NeuronCore's view). `repeat`
    re-runs the whole computation N times inside one NEFF (used to
    measure device time net of dispatch overhead)."""
    import concourse.bacc as bacc
    import concourse.mybir as mybir
    import concourse.tile as tile

    f32 = mybir.dt.float32
    f32r = mybir.dt.float32r
    bf16 = mybir.dt.bfloat16
    Alu = mybir.AluOpType
    Act = mybir.ActivationFunctionType

    n_ib = s // IB           # query blocks
    n_st = s // 128          # 128-row sequence tiles
    n_dt = D // 128          # contraction tiles for D
    n_pt = 256 // 128        # contraction tiles for the local 256 chans

    # Bacc (not plain Bass): its compile() lowers multi-wait sync_infos into
    # event-semaphore nops, which walrus codegen requires.
    nc = bacc.Bacc(num_devices=N_CORES)

    xt = nc.declare_dram_parameter("xt", [D, s], f32r, isOutput=False)
    wqk = nc.declare_dram_parameter("wqk", [D, 512], f32r, isOutput=False)
    wv = nc.declare_dram_parameter("wv", [D, 256], f32r, isOutput=False)
    bqk = nc.declare_dram_parameter("bqk", [128, 4], f32, isOutput=False)
    bv = nc.declare_dram_parameter("bv", [128, 256], f32, isOutput=False)
    # local 256 rows of w_proj, full output width
    wp = nc.declare_dram_parameter("wp", [256, D], bf16, isOutput=False)
    # b_proj / GROUP broadcast over partitions (folded into the psum bounce)
    bp = nc.declare_dram_parameter("bp", [128, D], f32, isOutput=False)
    msk = nc.declare_dram_parameter("msk", [128, 4, IB], bf16, isOutput=False)
    # all-ones constants (f32r memset fails the walrus ISA check, so DMA them)
    one_b = nc.declare_dram_parameter("one_b", [128, 4], bf16, isOutput=False)
    one_r = nc.declare_dram_parameter("one_r", [1, 128], f32r, isOutput=False)
    # selector for the pair-merged reciprocal replicate:
    # sel2[0,0:64]=1, sel2[1,64:128]=1
    sel2 = nc.declare_dram_parameter("sel2", [2, 128], f32r, isOutput=False)
    # bf16 reduce wire: halves collective bytes; host casts y back to f32
    y = nc.declare_dram_parameter("y", [s // GROUP, D], bf16, isOutput=True)

    with tile.TileContext(nc) as tc:
        with (
            tc.tile_pool(name="const", bufs=1) as const,
            tc.tile_pool(name="persist", bufs=1) as persist,
            tc.tile_pool(name="dram", bufs=1, space="DRAM") as dram,
        ):
            bqk_sb = const.tile([128, 4], f32)
            nc.sync.dma_start(out=bqk_sb, in_=bqk[:, :])
            bv_sb = const.tile([128, 256], f32)
            nc.sync.dma_start(out=bv_sb, in_=bv[:, :])
            # msk/wp/bp are not needed until well into the attention phase —
            # their loads are deferred below the first xt chunks
            bp_sb = const.tile([128, D], f32)
            msk_sb = const.tile([128, 4, IB], bf16)
            wp_sb = const.tile([128, n_pt, D], bf16)
            one_r_sb = const.tile([1, 128], f32r)
            nc.sync.dma_start(out=one_r_sb, in_=one_r[:, :])
            sel2_sb = const.tile([2, 128], f32r)
            nc.sync.dma_start(out=sel2_sb, in_=sel2[:, :])
            # dummy exp: pulls the ACT exp table load off the critical path
            warm_sb = const.tile([1, 1], f32)
            nc.scalar.activation(
                out=warm_sb, in_=bqk_sb[0:1, 0:1], func=Act.Exp, scale=0.0
            )

            # persistent intermediates
            for _rep in range(repeat):
             qT_sb = persist.tile([128, NPAIR, s], f32r, name="qT_sb")   # [pair_ch, pair, s]
             kT_sb = persist.tile([128, NPAIR, s], f32r)
             v_sb = persist.tile([128, n_st, HLOC, 65], bf16)  # [:, st, h, 64]=ones
             aT_sb = persist.tile([128, NPAIR, s], bf16)

             rs_in = dram.tile([s, D], bf16, name="rs_in")
             # collectives cannot write IO tensors: reduce into a DRAM
             # bounce, then DMA each chunk into the y output
             rs_out = dram.tile([s // GROUP, D], bf16, name="rs_out")

             for st in range(n_st):
                 nc.sync.dma_start(
                     out=v_sb[:, st, :, 64:65],
                     in_=one_b[:, 0:HLOC].rearrange("p (h o) -> p h o", o=1),
                 )

             # ------- phase 1 + 2: QKV projection interleaved with attention.
             with (
                 tc.tile_pool(name="p1in", bufs=1) as p1in,
             ):
                 # weights first: qkT/v matmuls need ALL of wqk/wv but only
                 # the first sequence-half of xt to get started. wqk split
                 # per c-tile so qkT(0) starts before the rest lands.
                 wqk_sb = p1in.tile([128, n_dt, 512], f32r)
                 for t4 in range(4):
                     for dh in range(2):
                         ds = slice(dh * n_dt // 2, (dh + 1) * n_dt // 2)
                         nc.sync.dma_start(
                             out=wqk_sb[:, ds, t4 * 128 : (t4 + 1) * 128],
                             in_=wqk.rearrange("(t p) c -> p t c", p=128)[
                                 :, ds, t4 * 128 : (t4 + 1) * 128
                             ],
                         )
                 wv_sb = p1in.tile([128, n_dt, 256], f32r)
                 for t4 in range(2):
                     nc.sync.dma_start(
                         out=wv_sb[:, :, t4 * 128 : (t4 + 1) * 128],
                         in_=wv.rearrange("(t p) c -> p t c", p=128)[
                             :, :, t4 * 128 : (t4 + 1) * 128
                         ],
                     )
                 # xt in two sequence-half tiles: in repeated execution the
                 # next iteration's first half can reload as soon as its
                 # last reader (early qkT/v blocks) is done, rather than
                 # waiting for the whole tensor's last reader
                 xt_shs = [
                     p1in.tile([128, n_dt, s // 2], f32r, name=f"xt{sh}")
                     for sh in range(2)
                 ]
                 for sh in range(2):
                     for t in range(n_dt):
                         for q in range(2):
                             # half-chunks spread across more DMA rings
                             # (real HW has 16; the model's 8 see no change)
                             qs = slice(q * s // 4, (q + 1) * s // 4)
                             nc.sync.dma_start(
                                 out=xt_shs[sh][:, t, qs],
                                 in_=xt.rearrange("(t p) ss -> p t ss", p=128)[
                                     :, t, sh * s // 2 + q * s // 4 :
                                     sh * s // 2 + (q + 1) * s // 4
                                 ],
                             )
                     if sh == 0 and _rep == 0:
                         # deferred const loads: needed only mid-attention
                         for q in range(2):
                             nc.sync.dma_start(
                                 out=msk_sb[:, 2 * q : 2 * q + 2, :],
                                 in_=msk[:, 2 * q : 2 * q + 2, :],
                             )
                             nc.sync.dma_start(
                                 out=wp_sb[:, q, :],
                                 in_=wp.rearrange(
                                     "(t p) c -> p t c", p=128
                                 )[:, q, :],
                             )
                             nc.sync.dma_start(
                                 out=bp_sb[:, q * D // 2 : (q + 1) * D // 2],
                                 in_=bp[:, q * D // 2 : (q + 1) * D // 2],
                             )

                 def xt_seq(dt, lo, width):
                     # [128, width] slice of transposed-x rows dt*128..,
                     # seq cols lo..lo+width (within one sequence half)
                     sh, off = divmod(lo, s // 2)
                     return xt_shs[sh][:, dt, off : off + width]

                 # v natural: lhsT = xT tile [d, s-tile], rhs = Wv [d, 256]
                 def v_for(st_lo, st_hi, pool, eng=None):
                     for st in range(st_lo, st_hi):
                         psv = pool.tile([128, 256], f32, name="psv", tag="pss")
                         for dt in range(n_dt):
                             nc.tensor.matmul(
                                 psv,
                                 lhsT=(xt_seq(dt, st * 128, 128)),
                                 rhs=(wv_sb[:, dt, :]),
                                 start=(dt == 0),
                                 stop=(dt == n_dt - 1),
                             )
                         (eng or nc.vector).tensor_tensor(
                             out=v_sb[:, st, :, 0:64],
                             in0=psv.rearrange("p (h e) -> p h e", h=HLOC),
                             in1=bv_sb.rearrange("p (h e) -> p h e", h=HLOC),
                             op=Alu.add,
                         )

                 def qkT_for(t, sb, pool):
                     # qT/kT: lhsT = W tile [d,c], rhs = xT [d, s-block]
                     # c-tile t: 0,1 = q pair0/1; 2,3 = k pair0/1
                     # bias add on ACT (idle outside the exp stream)
                     ps = pool.tile([128, IB], f32, name="ps", tag="pss")
                     for dt in range(n_dt):
                         nc.tensor.matmul(
                             ps,
                             lhsT=(wqk_sb[:, dt, t * 128 : (t + 1) * 128]),
                             rhs=(xt_seq(dt, sb * IB, IB)),
                             start=(dt == 0),
                             stop=(dt == n_dt - 1),
                         )
                     dst = qT_sb if t < 2 else kT_sb
                     nc.scalar.activation(
                         out=dst[:, t % 2, sb * IB : (sb + 1) * IB],
                         in_=ps,
                         func=Act.Identity,
                         bias=bqk_sb[:, t : t + 1],
                     )

                 if phases == "p1":
                     with tc.tile_pool(
                         name="ps_p1", bufs=2, space="PSUM"
                     ) as ps_p1:
                         v_for(0, n_st, ps_p1)
                         for sb in range(n_ib):
                             for t in range(4):
                                 qkT_for(t, sb, ps_p1)
                     continue
                 # ---- attention: head PAIRS, scores row-group packed ----
                 with (
                     tc.tile_pool(name="ps_s", bufs=2, space="PSUM") as ps_s,
                     tc.tile_pool(name="ps_av", bufs=1, space="PSUM") as ps_av,
                     tc.tile_pool(name="pt", bufs=4) as ptpool,
                     tc.tile_pool(name="small", bufs=4) as small,
                     tc.tile_pool(name="yout", bufs=3) as yout,
                 ):

                  def proj_for(ib, ypool):
                      # partial output projection for seq rows
                      # [ib*IB, (ib+1)*IB): contraction over the local 256
                      # channels (both pairs); b_proj/GROUP folded into the
                      # psum->SBUF bounce (DMA and GPSIMD cannot read PSUM,
                      # ACT is exp-saturated: DVE it is)
                      beng = nc.vector
                      for st in range(4 * ib, 4 * ib + 4):
                          psY = ps_s.tile([128, 2 * IB], f32, name="psY", tag="pss")
                          for half in range(2):
                              hs = slice(half * IB, (half + 1) * IB)
                              for P in range(NPAIR):
                                  nc.tensor.matmul(
                                      psY[:, hs],
                                      lhsT=(aT_sb[:, P, st * 128 : (st + 1) * 128]),
                                      rhs=(wp_sb[:, P, hs]),
                                      start=(P == 0),
                                      stop=(P == NPAIR - 1),
                                  )
                          ysb = ypool.tile([128, D], bf16, name="ysb")
                          beng.tensor_tensor(
                              out=ysb, in0=psY, in1=bp_sb, op=Alu.add
                          )
                          nc.sync.dma_start(
                              out=rs_in[st * 128 : (st + 1) * 128, :], in_=ysb
                          )

                  # uneven chunks: big early chunks hide under the remaining
                  # attention; the last (exposed) chunk is small
                  if rs_chunks == 3:
                      rs_bounds = {1: (0, 2 * IB), 2: (2 * IB, 3 * IB),
                                   3: (3 * IB, 4 * IB)}
                  elif rs_chunks == 2:
                      rs_bounds = {1: (0, 2 * IB), 3: (2 * IB, 4 * IB)}
                  else:
                      rs_bounds = {3: (0, 4 * IB)}

                  def proj_and_rs(ib):
                      proj_for(ib, yout)
                      if ib in rs_bounds:
                          lo, hi = rs_bounds[ib]
                          nc.gpsimd.collective_compute(
                              "ReduceScatter",
                              Alu.add,
                              replica_groups=[[0, 1, 2, 3], [4, 5, 6, 7]],
                              ins=[rs_in[lo:hi, :]],
                              outs=[rs_out[lo // GROUP : hi // GROUP, :]],
                          )
                          nq = max(1, (hi - lo) // GROUP // 128)
                          for q in range(nq):
                              qlo = lo // GROUP + q * 128
                              qhi = min(hi // GROUP, qlo + 128)
                              nc.sync.dma_start(
                                  out=y[qlo:qhi, :],
                                  in_=rs_out[qlo:qhi, :],
                              )

                  # q/k for BOTH pairs upfront (sb-major so early blocks
                  # only need the first xt sequence-half)
                  for sb in range(n_ib):
                      for t in range(4):
                          qkT_for(t, sb, ps_s)

                  # Both pairs' jt streams INTERLEAVED: while one pair's exp
                  # runs on ACT, the PE works the other pair's scores/AV —
                  # the per-jt exp latency never stalls the PE. v tiles load
                  # once per ib and feed both pairs.
                  pending = None   # proj deferred one ib into the next stream
                  for ib in range(n_ib):
                     v_for(4 * ib, 4 * ib + 4, ps_s)
                     njt = 4 * (ib + 1)  # key tiles needed (j <= i)
                     avs = {
                         (pair, hh): ps_av.tile(
                             [65, IB], f32, name=f"av{pair}{hh}",
                             tag=f"av{pair}{hh}",
                         )
                         for pair in range(NPAIR)
                         for hh in range(2)
                     }
                     # diagonal key tiles first: their mask multiply then
                     # overlaps the long non-diagonal score/AV stream
                     jt_order = list(range(4 * ib, njt)) + list(range(4 * ib))
                     for jseq, jt in enumerate(jt_order):
                         for pair in range(NPAIR):
                             pss = ps_s.tile([128, 2 * IB], f32, name="pss")
                             for hh in range(2):
                                 off = hh * 64
                                 nc.tensor.matmul(
                                     pss[:, hh * IB : (hh + 1) * IB],
                                     lhsT=(kT_sb[
                                             off : off + 64,
                                             pair,
                                             jt * 128 : (jt + 1) * 128,
                                         ]
                                     ),
                                     rhs=(qT_sb[
                                             off : off + 64,
                                             pair,
                                             ib * IB : (ib + 1) * IB,
                                         ]
                                     ),
                                     start=True,
                                     stop=True,
                                     tile_position=(off, 0),
                                 )
                             pt = ptpool.tile([128, 2 * IB], bf16, name="pt")
                             nc.scalar.activation(
                                 out=pt, in_=pss, func=Act.Exp, scale=SCALE
                             )
                             k = jt - 4 * ib
                             for hh in range(2):
                                 if k >= 0:  # diagonal tile: causal mask
                                     # DVE only: Pool would queue these
                                     # behind the previous rep's collective
                                     nc.vector.tensor_tensor(
                                         out=pt[:, hh * IB : (hh + 1) * IB],
                                         in0=pt[:, hh * IB : (hh + 1) * IB],
                                         in1=msk_sb[:, k, :],
                                         op=Alu.mult,
                                     )
                                 nc.tensor.matmul(
                                     avs[pair, hh],
                                     lhsT=(v_sb[:, jt, pair * 2 + hh, :]),
                                     rhs=(pt[:, hh * IB : (hh + 1) * IB]),
                                     start=(jseq == 0),
                                     stop=(jseq == njt - 1),
                                 )
                     if phases == "all" and pending is not None:
                         # previous ib's partial projection + reduce, emitted
                         # after this ib's whole jt stream: maximum slack for
                         # the previous normalize chain to drain on DVE
                         proj_and_rs(pending)
                         pending = None
                     # normalize per head: aT = av[0:64] * (1 / av[64]).
                     # Both heads of a pair share ONE replicate matmul
                     # (sel2 maps rec row hh to partitions hh*64..) and one
                     # psum->SBUF bounce (gpsimd partition_broadcast hangs
                     # this runtime, so replicate via PE).
                     # (matmul psum output must start at partition 0 — the
                     # pair-merged replicate into one [128,·] psum fails the
                     # walrus ISA check, so replicate per head)
                     for pair in range(NPAIR):
                         for hh in range(2):
                             av = avs[pair, hh]
                             off = hh * 64
                             rec_sb = small.tile([1, IB], f32r, name="rec_sb")
                             with nc.allow_low_precision(
                                 reason="softmax denom reciprocal, f32r"
                             ):
                                 nc.vector.reciprocal(rec_sb, av[64:65, :])
                             rec_ps = ps_s.tile(
                                 [64, IB], f32, name="rec_ps", tag="pss"
                             )
                             nc.tensor.matmul(
                                 rec_ps,
                                 lhsT=(one_r_sb[0:1, 0:64]),
                                 rhs=(rec_sb[:, :]),
                                 start=True,
                                 stop=True,
                             )
                             rec_rep = small.tile(
                                 [64, IB], f32, name="rec_rep"
                             )
                             if pair == 0:
                                 nc.vector.tensor_copy(
                                     out=rec_rep, in_=rec_ps
                                 )
                             else:
                                 # ACT has slack at ib boundaries
                                 nc.scalar.activation(
                                     out=rec_rep, in_=rec_ps, func=Act.Copy
                                 )
                             nc.vector.tensor_tensor(
                                 out=aT_sb[
                                     off : off + 64, pair, ib * IB : (ib + 1) * IB
                                 ],
                                 in0=av[0:64, :],
                                 in1=rec_rep,
                                 op=Alu.mult,
                             )
                     pending = ib
                  if phases == "all" and pending is not None:
                      proj_and_rs(pending)

    nc.compile()
    return nc


def _shard_inputs(x, w_attn, b_attn, w_proj, b_proj, s=S):
    """Host-side sharding: build the per-core input maps."""
    import ml_dtypes
    x = np.asarray(x, dtype=np.float32)
    w_attn = np.asarray(w_attn, dtype=np.float32)
    b_attn = np.asarray(b_attn, dtype=np.float32)
    w_proj = np.asarray(w_proj, dtype=np.float32)
    b_proj = np.asarray(b_proj, dtype=np.float32)

    # causal mask tiles: msk[j, k, i] = 1.0 if i >= j + 128*k
    jj = np.arange(128)[:, None, None]
    kk = np.arange(4)[None, :, None]
    ii = np.arange(IB)[None, None, :]
    msk = (ii >= jj + 128 * kk).astype(ml_dtypes.bfloat16)

    in_maps = []
    for core in range(N_CORES):
        b, g = divmod(core, GROUP)
        hs = list(range(g * HLOC, (g + 1) * HLOC))
        xt = np.ascontiguousarray(x[b].T)
        qcols = np.concatenate(
            [w_attn[:, h * HD : (h + 1) * HD] for h in hs], axis=1
        )
        kcols = np.concatenate(
            [w_attn[:, D + h * HD : D + (h + 1) * HD] for h in hs], axis=1
        )
        vcols = np.concatenate(
            [w_attn[:, 2 * D + h * HD : 2 * D + (h + 1) * HD] for h in hs], axis=1
        )
        wqk = np.ascontiguousarray(np.concatenate([qcols, kcols], axis=1))
        wvv = np.ascontiguousarray(vcols)
        bq = np.concatenate([b_attn[h * HD : (h + 1) * HD] for h in hs])
        bk = np.concatenate([b_attn[D + h * HD : D + (h + 1) * HD] for h in hs])
        bvv = np.concatenate(
            [b_attn[2 * D + h * HD : 2 * D + (h + 1) * HD] for h in hs]
        )
        bqk = np.concatenate([bq, bk]).reshape(4, 128).T.copy()  # [128, 4]
        bv = np.broadcast_to(bvv, (128, 256)).copy()
        wpc = np.ascontiguousarray(
            w_proj[g * 256 : (g + 1) * 256, :]
        ).astype(ml_dtypes.bfloat16)
        bpc = np.broadcast_to(b_proj / GROUP, (128, D)).astype(np.float32).copy()
        in_maps.append(
            dict(
                xt=xt, wqk=wqk, wv=wvv, bqk=bqk, bv=bv, wp=wpc, bp=bpc,
                msk=msk,
                one_b=np.ones((128, 4), ml_dtypes.bfloat16),
                one_r=np.ones((1, 128), np.float32),
                sel2=np.repeat(np.eye(2, dtype=np.float32), 64, axis=1),
            )
        )
    return in_maps


_BOUNDS_BY_CHUNKS = {
    3: [(0, 2 * IB), (2 * IB, 3 * IB), (3 * IB, 4 * IB)],
    2: [(0, 2 * IB), (2 * IB, 4 * IB)],
    1: [(0, 4 * IB)],
}
RS_BOUNDS = _BOUNDS_BY_CHUNKS[RS_CHUNKS]


def _unshard(results):
    y = np.empty((B, S, D), np.float32)
    for core in range(N_CORES):
        b, g = divmod(core, GROUP)
        res = results[core]["y"]
        for lo, hi in RS_BOUNDS:
            n = (hi - lo) // GROUP
            y[b, lo + g * n : lo + (g + 1) * n, :] = res[
                lo // GROUP : hi // GROUP
            ]
    return y


_NC_CACHE = {}


def kernel(x, w_attn, b_attn, w_proj, b_proj):
    from concourse.bass_utils import run_bass_kernel_spmd

    if S not in _NC_CACHE:
        _NC_CACHE[S] = _build_bass(S)
    nc = _NC_CACHE[S]
    in_maps = _shard_inputs(x, w_attn, b_attn, w_proj, b_proj)
    res = run_bass_kernel_spmd(nc, in_maps, list(range(N_CORES)))
    return _unshard(res.results)


# revision 77
# speedup vs baseline: 1.1651x; 1.0868x over previous
"""Trainium2 Bass kernel for a GPT-style causal attention block.

  y = proj( softmax_causal( (x@Wq)(x@Wk)^T / sqrt(hd) ) @ (x@Wv) )

Shapes (hardcoded): B=2, S=2048, D=1024, H=16 heads, hd=64.

Sharding over 8 NeuronCores: core = (batch b, head-group g), g selects 4
heads. Each core:
  phase 1: QKV projection for its 4 heads (fp32r matmuls).
           q,k produced TRANSPOSED  [head_ch, S]  (contraction-ready),
           v produced natural       [S, head_ch] (+ a ones column), bf16.
           q/k bias adds run on the ACT engine (Identity + per-partition
           bias AP). x arrives pre-transposed in two sequence-half SBUF
           tiles so repeated executions can reload the first half early.
  phase 2: causal attention; BOTH head pairs' jt streams interleaved so
           the per-jt ACT exp latency never stalls the PE, in the
           transposed-score layout [key, query]: the two K=64 score
           matmuls of a pair run CONCURRENTLY in the PE array via
           row-group tile_position (0,0)/(64,0); exp on ACT (scale=1/8
           folded in) writes bf16; causal mask multiplies only the
           diagonal key tiles (bf16 on DVE, 2-4x element rate);
           AV matmul with lhsT=[v|1] bf16 so psum row 64 accumulates
           the softmax denominator; normalization: DVE reciprocal →
           PE-replicate matmul → DVE/ACT bounce → DVE multiply into
           bf16 aT (gpsimd partition_broadcast hangs this runtime
           build, and a pending collective blocks the gpsimd queue).
  phase 3: head/tensor-parallel output projection via ReduceScatter:
           each core computes the PARTIAL projection of its own 256
           channels over the FULL output width, deferred one query
           block so it fills PE bubbles under the next exp stream;
           b_proj/GROUP is folded into the psum->SBUF bounce (DVE,
           bf16); a single ReduceScatter(add) across the 4 cores of
           the batch sums the partials and leaves each core its
           sequence shard of y (collectives cannot write IO tensors,
           so it lands in a DRAM bounce DMA'd to y). One big reduce
           beats chunked overlap on real HW: each collective carries
           ~11us more constant overhead than the cost model's 15us,
           and in repeated execution the tail reduce overlaps the next
           iteration's compute anyway.

Matmuls run in float32r (full PE rate when the moving free dim is
>=256); attention-weight/V/proj matmuls in bf16 (same PE rate). All
host-side sharding/layout prep is data-only so the single SPMD program
is rank-independent.
"""

import numpy as np

B = 2
S = 2048
D = 1024
H = 16
HD = 64
HLOC = 4          # heads per core
NPAIR = 2         # head pairs per core
N_CORES = 8
GROUP = 4         # cores per batch (replica group size)
IB = 512          # query block width (matmul moving dim)
JT = 128          # key tile (psum partition dim)
SCALE = 1.0 / 8.0  # 1/sqrt(hd)
RS_CHUNKS = 1     # ReduceScatter chunks (HW: each collective has ~11us more
                  # constant overhead than the cost model's 15us — one big
                  # reduce beats chunked overlap in steady state)


def _build_bass(s=S, repeat=1, phases="all", rs_chunks=RS_CHUNKS):
    """Build the SPMD Bass program (one NeuronCore's view). `repeat`
    re-runs the whole computation N times inside one NEFF (used to
    measure device time net of dispatch overhead)."""
    import concourse.bacc as bacc
    import concourse.mybir as mybir
    import concourse.tile as tile

    f32 = mybir.dt.float32
    f32r = mybir.dt.float32r
    bf16 = mybir.dt.bfloat16
    Alu = mybir.AluOpType
    Act = mybir.ActivationFunctionType

    n_ib = s // IB           # query blocks
    n_st = s // 128          # 128-row sequence tiles
    n_dt = D // 128          # contraction tiles for D
    n_pt = 256 // 128        # contraction tiles for the local 256 chans

    # Bacc (not plain Bass): its compile() lowers multi-wait sync_infos into
    # event-semaphore nops, which walrus codegen requires.
    nc = bacc.Bacc(num_devices=N_CORES)

    xt = nc.declare_dram_parameter("xt", [D, s], f32r, isOutput=False)
    wqk = nc.declare_dram_parameter("wqk", [D, 512], f32r, isOutput=False)
    wv = nc.declare_dram_parameter("wv", [D, 256], f32r, isOutput=False)
    bqk = nc.declare_dram_parameter("bqk", [128, 4], f32, isOutput=False)
    bv = nc.declare_dram_parameter("bv", [128, 256], f32, isOutput=False)
    # local 256 rows of w_proj, full output width
    wp = nc.declare_dram_parameter("wp", [256, D], bf16, isOutput=False)
    # b_proj / GROUP broadcast over partitions (folded into the psum bounce)
    bp = nc.declare_dram_parameter("bp", [128, D], f32, isOutput=False)
    msk = nc.declare_dram_parameter("msk", [128, 4, IB], bf16, isOutput=False)
    # all-ones constants (f32r memset fails the walrus ISA check, so DMA them)
    one_b = nc.declare_dram_parameter("one_b", [128, 4], bf16, isOutput=False)
    one_r = nc.declare_dram_parameter("one_r", [1, 128], f32r, isOutput=False)
    # selector for the pair-merged reciprocal replicate:
    # sel2[0,0:64]=1, sel2[1,64:128]=1
    sel2 = nc.declare_dram_parameter("sel2", [2, 128], f32r, isOutput=False)
    # bf16 reduce wire: halves collective bytes; host casts y back to f32
    y = nc.declare_dram_parameter("y", [s // GROUP, D], bf16, isOutput=True)

    with tile.TileContext(nc) as tc:
        with (
            tc.tile_pool(name="const", bufs=1) as const,
            tc.tile_pool(name="persist", bufs=1) as persist,
            tc.tile_pool(name="dram", bufs=1, space="DRAM") as dram,
        ):
            bqk_sb = const.tile([128, 4], f32)
            nc.sync.dma_start(out=bqk_sb, in_=bqk[:, :])
            bv_sb = const.tile([128, 256], f32)
            nc.sync.dma_start(out=bv_sb, in_=bv[:, :])
            # msk/wp/bp are not needed until well into the attention phase —
            # their loads are deferred below the first xt chunks
            bp_sb = const.tile([128, D], f32)
            msk_sb = const.tile([128, 4, IB], bf16)
            wp_sb = const.tile([128, n_pt, D], bf16)
            one_r_sb = const.tile([1, 128], f32r)
            nc.sync.dma_start(out=one_r_sb, in_=one_r[:, :])
            sel2_sb = const.tile([2, 128], f32r)
            nc.sync.dma_start(out=sel2_sb, in_=sel2[:, :])
            # dummy exp: pulls the ACT exp table load off the critical path
            warm_sb = const.tile([1, 1], f32)
            nc.scalar.activation(
                out=warm_sb, in_=bqk_sb[0:1, 0:1], func=Act.Exp, scale=0.0
            )

            # persistent intermediates
            for _rep in range(repeat):
             qT_sb = persist.tile([128, NPAIR, s], f32r, name="qT_sb")   # [pair_ch, pair, s]
             kT_sb = persist.tile([128, NPAIR, s], f32r)
             v_sb = persist.tile([128, n_st, HLOC, 65], bf16)  # [:, st, h, 64]=ones
             aT_sb = persist.tile([128, NPAIR, s], bf16)

             rs_in = dram.tile([s, D], bf16, name="rs_in")
             # collectives cannot write IO tensors: reduce into a DRAM
             # bounce, then DMA each chunk into the y output
             rs_out = dram.tile([s // GROUP, D], bf16, name="rs_out")

             for st in range(n_st):
                 nc.sync.dma_start(
                     out=v_sb[:, st, :, 64:65],
                     in_=one_b[:, 0:HLOC].rearrange("p (h o) -> p h o", o=1),
                 )

             # ------- phase 1 + 2: QKV projection interleaved with attention.
             with (
                 tc.tile_pool(name="p1in", bufs=1) as p1in,
             ):
                 # weights first: qkT/v matmuls need ALL of wqk/wv but only
                 # the first sequence-half of xt to get started. wqk split
                 # per c-tile so qkT(0) starts before the rest lands.
                 wqk_sb = p1in.tile([128, n_dt, 512], f32r)
                 for t4 in range(4):
                     for dh in range(2):
                         ds = slice(dh * n_dt // 2, (dh + 1) * n_dt // 2)
                         nc.sync.dma_start(
                             out=wqk_sb[:, ds, t4 * 128 : (t4 + 1) * 128],
                             in_=wqk.rearrange("(t p) c -> p t c", p=128)[
                                 :, ds, t4 * 128 : (t4 + 1) * 128
                             ],
                         )
                 wv_sb = p1in.tile([128, n_dt, 256], f32r)
                 for t4 in range(2):
                     nc.sync.dma_start(
                         out=wv_sb[:, :, t4 * 128 : (t4 + 1) * 128],
                         in_=wv.rearrange("(t p) c -> p t c", p=128)[
                             :, :, t4 * 128 : (t4 + 1) * 128
                         ],
                     )
                 # xt in two sequence-half tiles: in repeated execution the
                 # next iteration's first half can reload as soon as its
                 # last reader (early qkT/v blocks) is done, rather than
                 # waiting for the whole tensor's last reader
                 xt_shs = [
                     p1in.tile([128, n_dt, s // 2], f32r, name=f"xt{sh}")
                     for sh in range(2)
                 ]
                 for sh in range(2):
                     for t in range(n_dt):
                         for q in range(2):
                             # half-chunks spread across more DMA rings
                             # (real HW has 16; the model's 8 see no change)
                             qs = slice(q * s // 4, (q + 1) * s // 4)
                             nc.sync.dma_start(
                                 out=xt_shs[sh][:, t, qs],
                                 in_=xt.rearrange("(t p) ss -> p t ss", p=128)[
                                     :, t, sh * s // 2 + q * s // 4 :
                                     sh * s // 2 + (q + 1) * s // 4
                                 ],
                             )
                     if sh == 0 and _rep == 0:
                         # deferred const loads: needed only mid-attention
                         for q in range(2):
                             nc.sync.dma_start(
                                 out=msk_sb[:, 2 * q : 2 * q + 2, :],
                                 in_=msk[:, 2 * q : 2 * q + 2, :],
                             )
                             nc.sync.dma_start(
                                 out=wp_sb[:, q, :],
                                 in_=wp.rearrange(
                                     "(t p) c -> p t c", p=128
                                 )[:, q, :],
                             )
                             nc.sync.dma_start(
                                 out=bp_sb[:, q * D // 2 : (q + 1) * D // 2],
                                 in_=bp[:, q * D // 2 : (q + 1) * D // 2],
                             )

                 def xt_seq(dt, lo, width):
                     # [128, width] slice of transposed-x rows dt*128..,
                     # seq cols lo..lo+width (within one sequence half)
                     sh, off = divmod(lo, s // 2)
                     return xt_shs[sh][:, dt, off : off + width]

                 # v natural: lhsT = xT tile [d, s-tile], rhs = Wv [d, 256]
                 def v_for(st_lo, st_hi, pool, eng=None):
                     for st in range(st_lo, st_hi):
                         psv = pool.tile([128, 256], f32, name="psv", tag="pss")
                         for dt in range(n_dt):
                             nc.tensor.matmul(
                                 psv,
                                 lhsT=(xt_seq(dt, st * 128, 128)),
                                 rhs=(wv_sb[:, dt, :]),
                                 start=(dt == 0),
                                 stop=(dt == n_dt - 1),
                             )
                         (eng or nc.vector).tensor_tensor(
                             out=v_sb[:, st, :, 0:64],
                             in0=psv.rearrange("p (h e) -> p h e", h=HLOC),
                             in1=bv_sb.rearrange("p (h e) -> p h e", h=HLOC),
                             op=Alu.add,
                         )

                 def qkT_for(t, sb, pool):
                     # qT/kT: lhsT = W tile [d,c], rhs = xT [d, s-block]
                     # c-tile t: 0,1 = q pair0/1; 2,3 = k pair0/1
                     # bias add on ACT (idle outside the exp stream)
                     ps = pool.tile([128, IB], f32, name="ps", tag="pss")
                     for dt in range(n_dt):
                         nc.tensor.matmul(
                             ps,
                             lhsT=(wqk_sb[:, dt, t * 128 : (t + 1) * 128]),
                             rhs=(xt_seq(dt, sb * IB, IB)),
                             start=(dt == 0),
                             stop=(dt == n_dt - 1),
                         )
                     dst = qT_sb if t < 2 else kT_sb
                     nc.scalar.activation(
                         out=dst[:, t % 2, sb * IB : (sb + 1) * IB],
                         in_=ps,
                         func=Act.Identity,
                         bias=bqk_sb[:, t : t + 1],
                     )

                 if phases == "p1":
                     with tc.tile_pool(
                         name="ps_p1", bufs=2, space="PSUM"
                     ) as ps_p1:
                         v_for(0, n_st, ps_p1)
                         for sb in range(n_ib):
                             for t in range(4):
                                 qkT_for(t, sb, ps_p1)
                     continue
                 # ---- attention: head PAIRS, scores row-group packed ----
                 with (
                     tc.tile_pool(name="ps_s", bufs=2, space="PSUM") as ps_s,
                     tc.tile_pool(name="ps_av", bufs=1, space="PSUM") as ps_av,
                     tc.tile_pool(name="pt", bufs=4) as ptpool,
                     tc.tile_pool(name="small", bufs=4) as small,
                     tc.tile_pool(name="yout", bufs=3) as yout,
                 ):

                  def proj_for(ib, ypool):
                      # partial output projection for seq rows
                      # [ib*IB, (ib+1)*IB): contraction over the local 256
                      # channels (both pairs); b_proj/GROUP folded into the
                      # psum->SBUF bounce (DMA and GPSIMD cannot read PSUM,
                      # ACT is exp-saturated: DVE it is)
                      beng = nc.vector
                      for st in range(4 * ib, 4 * ib + 4):
                          psY = ps_s.tile([128, 2 * IB], f32, name="psY", tag="pss")
                          for half in range(2):
                              hs = slice(half * IB, (half + 1) * IB)
                              for P in range(NPAIR):
                                  nc.tensor.matmul(
                                      psY[:, hs],
                                      lhsT=(aT_sb[:, P, st * 128 : (st + 1) * 128]),
                                      rhs=(wp_sb[:, P, hs]),
                                      start=(P == 0),
                                      stop=(P == NPAIR - 1),
                                  )
                          ysb = ypool.tile([128, D], bf16, name="ysb")
                          beng.tensor_tensor(
                              out=ysb, in0=psY, in1=bp_sb, op=Alu.add
                          )
                          nc.sync.dma_start(
                              out=rs_in[st * 128 : (st + 1) * 128, :], in_=ysb
                          )

                  # uneven chunks: big early chunks hide under the remaining
                  # attention; the last (exposed) chunk is small
                  if rs_chunks == 3:
                      rs_bounds = {1: (0, 2 * IB), 2: (2 * IB, 3 * IB),
                                   3: (3 * IB, 4 * IB)}
                  elif rs_chunks == 2:
                      rs_bounds = {1: (0, 2 * IB), 3: (2 * IB, 4 * IB)}
                  else:
                      rs_bounds = {3: (0, 4 * IB)}

                  def proj_and_rs(ib):
                      proj_for(ib, yout)
                      if ib in rs_bounds:
                          lo, hi = rs_bounds[ib]
                          nc.gpsimd.collective_compute(
                              "ReduceScatter",
                              Alu.add,
                              replica_groups=[[0, 1, 2, 3], [4, 5, 6, 7]],
                              ins=[rs_in[lo:hi, :]],
                              outs=[rs_out[lo // GROUP : hi // GROUP, :]],
                          )
                          nq = max(1, (hi - lo) // GROUP // 128)
                          for q in range(nq):
                              qlo = lo // GROUP + q * 128
                              qhi = min(hi // GROUP, qlo + 128)
                              nc.sync.dma_start(
                                  out=y[qlo:qhi, :],
                                  in_=rs_out[qlo:qhi, :],
                              )

                  # q/k for BOTH pairs upfront, sb-major: early blocks only
                  # need the first xt half, and in repeated execution this
                  # whole block is PE filler overlapping the previous
                  # iteration's tail collective (streaming qkT into the ib
                  # loop instead delays the proj->reduce chain and loses
                  # ~8us/iter of cross-iteration overlap)
                  for sb in range(n_ib):
                      for t in range(4):
                          qkT_for(t, sb, ps_s)

                  # Both pairs' jt streams INTERLEAVED: while one pair's exp
                  # runs on ACT, the PE works the other pair's scores/AV —
                  # the per-jt exp latency never stalls the PE. v tiles load
                  # once per ib and feed both pairs.
                  pending = None   # proj deferred one ib into the next stream
                  for ib in range(n_ib):
                     v_for(4 * ib, 4 * ib + 4, ps_s)
                     njt = 4 * (ib + 1)  # key tiles needed (j <= i)
                     avs = {
                         (pair, hh): ps_av.tile(
                             [65, IB], f32, name=f"av{pair}{hh}",
                             tag=f"av{pair}{hh}",
                         )
                         for pair in range(NPAIR)
                         for hh in range(2)
                     }
                     # diagonal key tiles first: their mask multiply then
                     # overlaps the long non-diagonal score/AV stream
                     jt_order = list(range(4 * ib, njt)) + list(range(4 * ib))
                     for jseq, jt in enumerate(jt_order):
                         for pair in range(NPAIR):
                             pss = ps_s.tile([128, 2 * IB], f32, name="pss")
                             for hh in range(2):
                                 off = hh * 64
                                 nc.tensor.matmul(
                                     pss[:, hh * IB : (hh + 1) * IB],
                                     lhsT=(kT_sb[
                                             off : off + 64,
                                             pair,
                                             jt * 128 : (jt + 1) * 128,
                                         ]
                                     ),
                                     rhs=(qT_sb[
                                             off : off + 64,
                                             pair,
                                             ib * IB : (ib + 1) * IB,
                                         ]
                                     ),
                                     start=True,
                                     stop=True,
                                     tile_position=(off, 0),
                                 )
                             pt = ptpool.tile([128, 2 * IB], bf16, name="pt")
                             nc.scalar.activation(
                                 out=pt, in_=pss, func=Act.Exp, scale=SCALE
                             )
                             k = jt - 4 * ib
                             for hh in range(2):
                                 if k >= 0:  # diagonal tile: causal mask
                                     # DVE only: Pool would queue these
                                     # behind the previous rep's collective
                                     nc.vector.tensor_tensor(
                                         out=pt[:, hh * IB : (hh + 1) * IB],
                                         in0=pt[:, hh * IB : (hh + 1) * IB],
                                         in1=msk_sb[:, k, :],
                                         op=Alu.mult,
                                     )
                                 nc.tensor.matmul(
                                     avs[pair, hh],
                                     lhsT=(v_sb[:, jt, pair * 2 + hh, :]),
                                     rhs=(pt[:, hh * IB : (hh + 1) * IB]),
                                     start=(jseq == 0),
                                     stop=(jseq == njt - 1),
                                 )
                     if phases == "all" and pending is not None:
                         # previous ib's partial projection + reduce, emitted
                         # after this ib's whole jt stream: maximum slack for
                         # the previous normalize chain to drain on DVE
                         proj_and_rs(pending)
                         pending = None
                     # normalize per head: aT = av[0:64] * (1 / av[64]).
                     # Both heads of a pair share ONE replicate matmul
                     # (sel2 maps rec row hh to partitions hh*64..) and one
                     # psum->SBUF bounce (gpsimd partition_broadcast hangs
                     # this runtime, so replicate via PE).
                     # (matmul psum output must start at partition 0 — the
                     # pair-merged replicate into one [128,·] psum fails the
                     # walrus ISA check, so replicate per head)
                     for pair in range(NPAIR):
                         for hh in range(2):
                             av = avs[pair, hh]
                             off = hh * 64
                             rec_sb = small.tile([1, IB], f32r, name="rec_sb")
                             with nc.allow_low_precision(
                                 reason="softmax denom reciprocal, f32r"
                             ):
                                 nc.vector.reciprocal(rec_sb, av[64:65, :])
                             rec_ps = ps_s.tile(
                                 [64, IB], f32, name="rec_ps", tag="pss"
                             )
                             nc.tensor.matmul(
                                 rec_ps,
                                 lhsT=(one_r_sb[0:1, 0:64]),
                                 rhs=(rec_sb[:, :]),
                                 start=True,
                                 stop=True,
                             )
                             rec_rep = small.tile(
                                 [64, IB], f32, name="rec_rep"
                             )
                             if pair == 0:
                                 nc.vector.tensor_copy(
                                     out=rec_rep, in_=rec_ps
                                 )
                             else:
                                 # ACT has slack at ib boundaries
                                 nc.scalar.activation(
                                     out=rec_rep, in_=rec_ps, func=Act.Copy
                                 )
                             nc.vector.tensor_tensor(
                                 out=aT_sb[
                                     off : off + 64, pair, ib * IB : (ib + 1) * IB
                                 ],
                                 in0=av[0:64, :],
                                 in1=rec_rep,
                                 op=Alu.mult,
                             )
                     pending = ib
                  if phases == "all" and pending is not None:
                      proj_and_rs(pending)

    nc.compile()
    return nc


def _shard_inputs(x, w_attn, b_attn, w_proj, b_proj, s=S):
    """Host-side sharding: build the per-core input maps."""
    import ml_dtypes
    x = np.asarray(x, dtype=np.float32)
    w_attn = np.asarray(w_attn, dtype=np.float32)
    b_attn = np.asarray(b_attn, dtype=np.float32)
    w_proj = np.asarray(w_proj, dtype=np.float32)
    b_proj = np.asarray(b_proj, dtype=np.float32)

    # causal mask tiles: msk[j, k, i] = 1.0 if i >= j + 128*k
    jj = np.arange(128)[:, None, None]
    kk = np.arange(4)[None, :, None]
    ii = np.arange(IB)[None, None, :]
    msk = (ii >= jj + 128 * kk).astype(ml_dtypes.bfloat16)

    in_maps = []
    for core in range(N_CORES):
        b, g = divmod(core, GROUP)
        hs = list(range(g * HLOC, (g + 1) * HLOC))
        xt = np.ascontiguousarray(x[b].T)
        qcols = np.concatenate(
            [w_attn[:, h * HD : (h + 1) * HD] for h in hs], axis=1
        )
        kcols = np.concatenate(
            [w_attn[:, D + h * HD : D + (h + 1) * HD] for h in hs], axis=1
        )
        vcols = np.concatenate(
            [w_attn[:, 2 * D + h * HD : 2 * D + (h + 1) * HD] for h in hs], axis=1
        )
        wqk = np.ascontiguousarray(np.concatenate([qcols, kcols], axis=1))
        wvv = np.ascontiguousarray(vcols)
        bq = np.concatenate([b_attn[h * HD : (h + 1) * HD] for h in hs])
        bk = np.concatenate([b_attn[D + h * HD : D + (h + 1) * HD] for h in hs])
        bvv = np.concatenate(
            [b_attn[2 * D + h * HD : 2 * D + (h + 1) * HD] for h in hs]
        )
        bqk = np.concatenate([bq, bk]).reshape(4, 128).T.copy()  # [128, 4]
        bv = np.broadcast_to(bvv, (128, 256)).copy()
        wpc = np.ascontiguousarray(
            w_proj[g * 256 : (g + 1) * 256, :]
        ).astype(ml_dtypes.bfloat16)
        bpc = np.broadcast_to(b_proj / GROUP, (128, D)).astype(np.float32).copy()
        in_maps.append(
            dict(
                xt=xt, wqk=wqk, wv=wvv, bqk=bqk, bv=bv, wp=wpc, bp=bpc,
                msk=msk,
                one_b=np.ones((128, 4), ml_dtypes.bfloat16),
                one_r=np.ones((1, 128), np.float32),
                sel2=np.repeat(np.eye(2, dtype=np.float32), 64, axis=1),
            )
        )
    return in_maps


_BOUNDS_BY_CHUNKS = {
    3: [(0, 2 * IB), (2 * IB, 3 * IB), (3 * IB, 4 * IB)],
    2: [(0, 2 * IB), (2 * IB, 4 * IB)],
    1: [(0, 4 * IB)],
}
RS_BOUNDS = _BOUNDS_BY_CHUNKS[RS_CHUNKS]


def _unshard(results):
    y = np.empty((B, S, D), np.float32)
    for core in range(N_CORES):
        b, g = divmod(core, GROUP)
        res = results[core]["y"]
        for lo, hi in RS_BOUNDS:
            n = (hi - lo) // GROUP
            y[b, lo + g * n : lo + (g + 1) * n, :] = res[
                lo // GROUP : hi // GROUP
            ]
    return y


_NC_CACHE = {}


def kernel(x, w_attn, b_attn, w_proj, b_proj):
    from concourse.bass_utils import run_bass_kernel_spmd

    if S not in _NC_CACHE:
        _NC_CACHE[S] = _build_bass(S)
    nc = _NC_CACHE[S]
    in_maps = _shard_inputs(x, w_attn, b_attn, w_proj, b_proj)
    res = run_bass_kernel_spmd(nc, in_maps, list(range(N_CORES)))
    return _unshard(res.results)
